# revision 8
# baseline (speedup 1.0000x reference)
"""GCN node classifier on 8 Trainium2 NeuronCores — v2.

3-layer GCN, N=100000 nodes, E=3.2M edges, d_in=512, d_h=32, n_cls=40.

Math refactor (linearity of gcn_conv):
    out_k = (dinv * (segsum(t_k[src]) + t_k)) @ W_k + b_k,  t_k = dinv * h_k
so gathers move 32-wide rows and per-edge norm becomes node-level dinv.

v2 structure vs v1:
  - gather reads the COMPACT [N,32] bf16 table directly with 256B elems
    (4 nodes per elem, idx = src>>2 fits int16); edges are quarter-sorted
    (src&3) per dst window so each 128-row subtile selects one 32-col
    quarter of the gathered elem. No padded-table expansion, no chunking.
  - one-hot scatter masks built with ONE batched DVE is_equal per window
    (98/layer instead of ~3.6k ops).
  - the inter-layer exchange sends RAW (un-normalized) u; the pairnorm
    stats AllReduce runs concurrently with the AllGather (Shared output),
    and each core normalizes the received full table locally.
  - W1/Wf applied via 4-slice block-diagonal matmuls (25/layer).
"""

import math
import numpy as np
import ml_dtypes

BF16 = ml_dtypes.bfloat16

# ---------------------------------------------------------------- config

class Cfg:
    def __init__(self, n_nodes, n_edges, d_in=512, d_h=32, n_cls=40, n_cores=8):
        assert n_nodes % n_cores == 0 and n_nodes % 4 == 0
        self.N = n_nodes
        self.E = n_edges
        self.C = n_cores
        self.NL = n_nodes // n_cores          # nodes per core
        self.TN = math.ceil(self.NL / 128)    # dst windows per core
        self.NLP = self.TN * 128
        self.TNF = math.ceil(n_nodes / 128)   # windows in the full table
        self.D_IN = d_in
        self.DH = d_h
        self.DC = n_cls
        self.EPS = 1e-5
        self.SUB = 2048                       # idxs per dma_gather call

FULL = Cfg(100000, 3200000)

# ------------------------------------------------------- host preprocessing

def preprocess(cfg, edge_index):
    """Quarter-sorted per-core gather indices + segment labels.

    Per core, edges sorted by (dst window w, src&3 quarter q); each
    (w, q) segment padded to R_q rows (global max, multiple of 128).
    Gather idx = src>>2 (256B elems of the compact [N,32] bf16 table).

    Returns dict with RS=(R_0..R_3) plus per-core tensors.
    """
    N, C, NL, TN = cfg.N, cfg.C, cfg.NL, cfg.TN
    src = np.asarray(edge_index[0], dtype=np.int64)
    dst = np.asarray(edge_index[1], dtype=np.int64)
    deg = np.bincount(dst, minlength=N).astype(np.float64) + 1.0  # + self loop
    dinv = (1.0 / np.sqrt(deg)).astype(np.float32)

    core = dst // NL
    nl = dst - core * NL
    w = nl >> 7
    dst_rel = nl & 127
    q = (src & 3).astype(np.int64)
    idxv = (src >> 2).astype(np.int16)

    gid = ((core * TN + w) * 4 + q)
    n_gid = C * TN * 4
    cnt = np.bincount(gid, minlength=n_gid).reshape(C, TN, 4)
    RS = [max(128, ((int(cnt[:, :, qq].max()) + 127) // 128) * 128)
          for qq in range(4)]
    SR = sum(RS)
    qoff = np.concatenate([[0], np.cumsum(RS)])[:4]

    order = np.argsort(gid, kind="stable")
    gid_s = gid[order]
    starts = np.zeros(n_gid + 1, dtype=np.int64)
    np.cumsum(cnt.reshape(-1), out=starts[1:])
    rank = np.arange(len(order), dtype=np.int64) - starts[gid_s]

    core_s = gid_s // (TN * 4)
    rem = gid_s - core_s * (TN * 4)
    w_s = rem >> 2
    q_s = rem & 3
    pos = w_s * SR + qoff[q_s] + rank

    rows_per_core = TN * SR
    idx_flat = np.zeros((C, rows_per_core), dtype=np.int16)
    seg_flat = np.full((C, rows_per_core), -1.0, dtype=np.float32)
    idx_flat[core_s, pos] = idxv[order]
    seg_flat[core_s, pos] = dst_rel[order]

    # 16-wrap per window block, replicate to 128 partitions
    gidx = (idx_flat.reshape(C, TN, SR // 16, 16)
            .transpose(0, 3, 1, 2).reshape(C, 16, TN * (SR // 16)))
    gidx = np.tile(gidx, (1, 8, 1))
    segrel = (seg_flat.reshape(C, TN, SR // 128, 128)
              .transpose(0, 3, 1, 2).reshape(C, 128, TN * (SR // 128)))

    dinv_nm = np.zeros((C, 128, TN), dtype=np.float32)
    for c in range(C):
        v = c * NL + np.arange(cfg.NLP)
        valid = v < (c + 1) * NL
        dd = np.where(valid, dinv[np.minimum(v, N - 1)], 0.0)
        dinv_nm[c] = dd.reshape(TN, 128).T
    # full-table dinv in [p, t] layout (node = t*128 + p)
    NF = cfg.TNF * 128
    vf = np.arange(NF)
    df = np.where(vf < N, dinv[np.minimum(vf, N - 1)], 0.0).astype(np.float32)
    dinv_full = df.reshape(cfg.TNF, 128).T.copy()

    return dict(RS=tuple(RS), gidx=np.ascontiguousarray(gidx),
                segrel=np.ascontiguousarray(segrel.astype(BF16)),
                dinv_nm=dinv_nm, dinv_full=dinv_full)


# ------------------------------------------------------------ device program

def build_program(cfg, RS, skip=frozenset(), reps01=1):
    """Build the SPMD bass program. Returns (nc, names).

    reps01 > 1 repeats the [layer0, layer1] pair for timing calibration
    (output is then mathematically meaningless).
    """
    from concourse import bass, bacc, mybir, tile
    from concourse.ap import AP as APc
    from concourse.masks import make_identity

    f32 = mybir.dt.float32
    bf16 = mybir.dt.bfloat16
    i16 = mybir.dt.int16

    N, NL, NLP, TN, TNF = cfg.N, cfg.NL, cfg.NLP, cfg.TN, cfg.TNF
    D_IN, DH, DC, SUB = cfg.D_IN, cfg.DH, cfg.DC, cfg.SUB
    SR = sum(RS)
    KW = SR // 128                       # subtiles per window
    SPW16 = SR // 16                     # idx cols per window
    # quarter of each subtile
    qmap = []
    for qq in range(4):
        qmap += [qq] * (RS[qq] // 128)
    NQUAD = TN // 4                      # full 4-slice quads (TN=98 -> 24)
    TTAIL = TN - NQUAD * 4               # leftover slices (2)
    KC = D_IN // 128
    rg = [list(range(cfg.C))]
    GSL = 8                              # windows per resident gidx slab
    NSL = math.ceil(TN / GSL)

    nc = bacc.Bacc("TRN2", target_bir_lowering=False, debug=False,
                   num_devices=cfg.C, num_swdge_queues=4,
                   dynamic_dma_scratch_size=32768)

    # -------- kernel I/O
    x_bf = nc.dram_tensor("x_bf", [NLP, D_IN], bf16, kind="ExternalInput").ap()
    gidx_d = nc.dram_tensor("gidx", [128, TN * SPW16], i16,
                            kind="ExternalInput").ap()
    segrel_d = nc.dram_tensor("segrel", [128, TN * KW], bf16,
                              kind="ExternalInput").ap()
    dinv_d = nc.dram_tensor("dinv_nm", [128, TN], f32, kind="ExternalInput").ap()
    dinvf_d = nc.dram_tensor("dinv_full", [128, TNF], f32,
                             kind="ExternalInput").ap()
    w0_d = nc.dram_tensor("w0", [D_IN, DH], bf16, kind="ExternalInput").ap()
    wblk1_d = nc.dram_tensor("wblk1", [128, 4 * DH], f32,
                             kind="ExternalInput").ap()
    wblkf_d = nc.dram_tensor("wblkf", [128, 4 * DC], f32,
                             kind="ExternalInput").ap()
    sel32_d = nc.dram_tensor("sel32", [128, DH], f32, kind="ExternalInput").ap()
    b0_d = nc.dram_tensor("b0", [128, DH], f32, kind="ExternalInput").ap()
    b1q_d = nc.dram_tensor("b1q", [128, 4 * DH], f32, kind="ExternalInput").ap()
    bfq_d = nc.dram_tensor("bfq", [128, 4 * DC], f32, kind="ExternalInput").ap()
    iota_d = nc.dram_tensor("iota_row", [128, 128], bf16,
                            kind="ExternalInput").ap()
    out_d = nc.dram_tensor("out", [NL, DC], f32, kind="ExternalOutput").ap()

    # -------- internal DRAM
    NF = TNF * 128                       # 128-padded full-table rows
    u_shard = nc.dram_tensor("u_shard", [NLP, DH], bf16).ap()
    u_full = nc.dram_tensor("u_full", [NF, DH], bf16,
                            addr_space="Shared").ap()
    tab_n = nc.dram_tensor("tab_n", [NF, DH], bf16).ap()
    st_in = nc.dram_tensor("st_in", [1, 64], f32).ap()
    st_out = nc.dram_tensor("st_out", [1, 64], f32).ap()

    # gather-source views: [(32, N/4... rows), (1, 128)] 256B elems
    NP4 = N // 4
    gsrc_u = APc(tensor=u_full.tensor, offset=0, ap=[[128, NP4], [1, 128]])
    gsrc_t = APc(tensor=tab_n.tensor, offset=0, ap=[[128, NP4], [1, 128]])

    names = ["x_bf", "gidx", "segrel", "dinv_nm", "dinv_full", "w0", "wblk1",
             "wblkf", "sel32", "b0", "b1q", "bfq", "iota_row"]

    with tile.TileContext(nc) as tc:
        import contextlib
        with contextlib.ExitStack() as ctx:
            big = ctx.enter_context(tc.tile_pool(name="big", bufs=1))
            gp = ctx.enter_context(tc.tile_pool(name="gp", bufs=2))
            msgp = ctx.enter_context(tc.tile_pool(name="msg", bufs=2))
            ohp = ctx.enter_context(tc.tile_pool(name="oh", bufs=2))
            smp = ctx.enter_context(tc.tile_pool(name="sm", bufs=4))
            usl = ctx.enter_context(tc.tile_pool(name="usl", bufs=2))
            psw = ctx.enter_context(tc.tile_pool(name="psw", bufs=4,
                                                 space="PSUM"))
            pstr = ctx.enter_context(tc.tile_pool(name="pstr", bufs=3,
                                                  space="PSUM"))
            psst = ctx.enter_context(tc.tile_pool(name="psst", bufs=1,
                                                  space="PSUM"))

            # ---- residents
            segrel_sb = big.tile([128, TN * KW], bf16)
            dinv_sb = big.tile([128, TN], f32)
            dinvf_sb = big.tile([128, TNF], f32)
            w0_sb = big.tile([128, D_IN // 128, DH], bf16)
            wblk1_sb = big.tile([128, 4 * DH], f32)
            wblkf_sb = big.tile([128, 4 * DC], f32)
            sel32_sb = big.tile([128, DH], f32)
            b0_sb = big.tile([128, DH], f32)
            b1q_sb = big.tile([128, 4 * DH], f32)
            bfq_sb = big.tile([128, 4 * DC], f32)
            iota_sb = big.tile([128, 128], bf16)
            ident = big.tile([128, 128], f32)
            ones_sb = big.tile([128, 1], f32)
            ones_row = big.tile([1, 128], f32)
            stats_sb = big.tile([1, 64], f32)
            s_sb = big.tile([1, 64], f32)

            tloc = big.tile([128, TN, DH], f32)     # t_k local (self term)
            agg = big.tile([128, TN, DH], f32)
            state = big.tile([128, TN, DH], f32)    # u_k
            shard_sb = big.tile([128, TN, DH], bf16)
            final_sb = big.tile([128, TN, DC], f32)
            agg2 = agg[:].rearrange("p t f -> p (t f)")
            state2 = state[:].rearrange("p t f -> p (t f)")
            final2 = final_sb[:].rearrange("p t f -> p (t f)")

            dinvfb_sb = big.tile([128, TNF], bf16)
            nc.sync.dma_start(out=segrel_sb[:], in_=segrel_d[:])
            nc.sync.dma_start(out=dinv_sb[:], in_=dinv_d[:])
            nc.sync.dma_start(out=dinvf_sb[:], in_=dinvf_d[:])
            nc.vector.tensor_copy(out=dinvfb_sb[:], in_=dinvf_sb[:])
            nc.sync.dma_start(out=w0_sb[:],
                              in_=w0_d.rearrange("(c p) f -> p c f", p=128))
            nc.sync.dma_start(out=wblk1_sb[:], in_=wblk1_d[:])
            nc.sync.dma_start(out=wblkf_sb[:], in_=wblkf_d[:])
            nc.sync.dma_start(out=sel32_sb[:], in_=sel32_d[:])
            nc.sync.dma_start(out=b0_sb[:], in_=b0_d[:])
            nc.sync.dma_start(out=b1q_sb[:], in_=b1q_d[:])
            nc.sync.dma_start(out=bfq_sb[:], in_=bfq_d[:])
            nc.sync.dma_start(out=iota_sb[:], in_=iota_d[:])
            make_identity(nc, ident[:])
            nc.vector.memset(ones_sb[:], 1.0)
            nc.vector.memset(ones_row[:], 1.0)

            def dinv_b(shape):
                return dinv_sb[:, :, None].to_broadcast(shape)

            # ---------------- phase A: tloc = dinv * (x @ W0)
            NB = 8
            TH0 = (TN + NB - 1) // NB
            bands = [] if "xw0" in skip else \
                [(b * TH0, min(TN, (b + 1) * TH0)) for b in range(NB)]
            with tc.tile_pool(name="xTp", bufs=1) as xTp:
                for h, (tlo, thi) in enumerate(bands):
                    nh = thi - tlo
                    if nh <= 0:
                        continue
                    xT = [xTp.tile([128, TH0 * 128], bf16, tag=f"xT{c}",
                                   name=f"xT{c}_{h}") for c in range(KC)]
                    for c in range(KC):
                        nc.sync.dma_start_transpose(
                            out=xT[c][:, 0:nh * 128],
                            in_=x_bf[tlo * 128: thi * 128,
                                     128 * c:128 * (c + 1)])
                    for t in range(tlo, thi):
                        t0 = pstr.tile([128, DH], f32, tag="small",
                                       name=f"t0_{t}")
                        for c in range(KC):
                            nc.tensor.matmul(
                                out=t0[:],
                                lhsT=xT[c][:, 128 * (t - tlo):128 * (t - tlo + 1)],
                                rhs=w0_sb[:, c, :],
                                start=(c == 0), stop=(c == KC - 1))
                        nc.vector.tensor_tensor(
                            out=tloc[:, t, :], in0=t0[:],
                            in1=dinv_sb[:, t:t + 1].to_broadcast([128, DH]),
                            op=mybir.AluOpType.mult)

            # exchange t0 (raw table for layer 0; no normalization needed)
            nc.vector.tensor_copy(out=shard_sb[:], in_=tloc[:])
            nc.sync.dma_start(out=u_shard.rearrange("(t p) f -> p t f", p=128),
                              in_=shard_sb[:])
            if "ag" not in skip:
                nc.gpsimd.collective_compute(
                    "AllGather", mybir.AluOpType.bypass, replica_groups=rg,
                    ins=[u_shard[0:NL, :]], outs=[u_full[0:N, :]])

            # ---------------- layers
            qrot = [0]
            layer_seq = [0, 1] * reps01 + [2]
            for step, layer in enumerate(layer_seq):
                gsrc = gsrc_u if step == 0 else gsrc_t
                # ---- gather + segment-sum into agg
                for sl in range(NSL):
                    g0, g1 = sl * GSL, min(TN, (sl + 1) * GSL)
                    gt = gp.tile([128, GSL * SPW16], i16, tag="gidx")
                    nc.sync.dma_start(
                        out=gt[:, 0:(g1 - g0) * SPW16],
                        in_=gidx_d[:, g0 * SPW16:g1 * SPW16])
                    for g in range(g0, g1):
                        lo = (g - g0) * SPW16
                        msg = (msgp.tile([128, KW, 128], bf16, tag="msg",
                                          name=f"msg_{step}_{g}")
                               if "gather" not in skip else None)
                        nsub = (SR + SUB - 1) // SUB
                        for j in range(nsub if "gather" not in skip else 0):
                            ni = min(SUB, SR - j * SUB)
                            nc.gpsimd.dma_gather(
                                out_ap=msg[:, j * (SUB // 128):
                                           j * (SUB // 128) + ni // 128, :],
                                in_ap=gsrc,
                                idxs_ap=gt[:, lo + j * (SUB // 16):
                                           lo + j * (SUB // 16) + ni // 16],
                                num_idxs=ni,
                                num_idxs_reg=ni,
                                elem_size=128,
                                single_packet=False,
                                queue_num=qrot[0] % 4,
                            )
                            qrot[0] += 1
                        oh = (ohp.tile([128, KW, 128], bf16, tag="oh",
                                        name=f"oh_{step}_{g}")
                              if "oneh" not in skip else None)
                        if "oneh" not in skip:
                            nc.vector.tensor_tensor(
                                out=oh[:],
                                in0=iota_sb[:, None, :]
                                    .to_broadcast([128, KW, 128]),
                                in1=segrel_sb[:, g * KW:(g + 1) * KW, None]
                                    .to_broadcast([128, KW, 128]),
                                op=mybir.AluOpType.is_equal)
                        pws = (psw.tile([128, DH], f32, tag="pw",
                                        name=f"pw_{step}_{g}")
                               if "scatter" not in skip else None)
                        if "scatter" not in skip:
                            for k in range(KW):
                                nc.tensor.matmul(
                                    out=pws[:],
                                    lhsT=oh[:, k, :],
                                    rhs=msg[:, k, qmap[k] * DH:
                                            (qmap[k] + 1) * DH],
                                    start=(k == 0), stop=(k == KW - 1))
                            nc.scalar.activation(
                                out=agg[:, g, :], in_=pws[:],
                                func=mybir.ActivationFunctionType.Copy)

                # ---- pre = dinv * (agg + tloc)   (into agg)
                nc.vector.tensor_tensor(out=agg[:], in0=agg[:], in1=tloc[:],
                                        op=mybir.AluOpType.add)
                nc.vector.tensor_tensor(out=agg[:], in0=agg[:],
                                        in1=dinv_b([128, TN, DH]),
                                        op=mybir.AluOpType.mult)

                # ---- u = pre @ W + b
                if layer == 0:
                    nc.vector.tensor_tensor(
                        out=state[:], in0=agg[:],
                        in1=b0_sb[:, None, :].to_broadcast([128, TN, DH]),
                        op=mybir.AluOpType.add)
                else:
                    W_sb, bq_sb, DO = ((wblk1_sb, b1q_sb, DH) if layer == 1
                                       else (wblkf_sb, bfq_sb, DC))
                    dst2 = state2 if layer == 1 else final2
                    quads = [(i * 4, 4) for i in range(NQUAD)]
                    if TTAIL:
                        quads.append((NQUAD * 4, TTAIL))
                    for t0q, nsl_ in quads:
                        ncol = nsl_ * DH
                        ptp = pstr.tile([128, 128], f32, tag="small",
                                        name=f"tr{step}_{t0q}")
                        nc.tensor.transpose(
                            out=ptp[0:ncol, :],
                            in_=agg2[:, t0q * DH:t0q * DH + ncol],
                            identity=ident[:])
                        preT = smp.tile([128, 128], f32, tag="preT")
                        nc.scalar.activation(
                            out=preT[0:ncol, :], in_=ptp[0:ncol, :],
                            func=mybir.ActivationFunctionType.Copy)
                        po = pstr.tile([128, 4 * DO], f32, tag="small",
                                       name=f"po{step}_{t0q}")
                        nc.tensor.matmul(out=po[:, 0:nsl_ * DO],
                                         lhsT=preT[0:ncol, :],
                                         rhs=W_sb[0:ncol, 0:nsl_ * DO],
                                         start=True, stop=True)
                        nc.vector.tensor_tensor(
                            out=dst2[:, t0q * DO:(t0q + nsl_) * DO],
                            in0=po[:, 0:nsl_ * DO],
                            in1=bq_sb[:, 0:nsl_ * DO],
                            op=mybir.AluOpType.add)

                if layer == 2:
                    nc.sync.dma_start(
                        out=out_d[0:(TN - 1) * 128, :].rearrange(
                            "(t p) f -> p t f", p=128),
                        in_=final_sb[:, 0:TN - 1, :])
                    lastn = NL - (TN - 1) * 128
                    nc.sync.dma_start(
                        out=out_d[(TN - 1) * 128: NL, :],
                        in_=final_sb[0:lastn, TN - 1, :])
                    continue

                # ---- exchange raw u || pairnorm stats
                nc.vector.tensor_copy(out=shard_sb[:], in_=state[:])
                nc.sync.dma_start(
                    out=u_shard.rearrange("(t p) f -> p t f", p=128),
                    in_=shard_sb[:])
                if "ag" not in skip:
                    nc.gpsimd.collective_compute(
                        "AllGather", mybir.AluOpType.bypass, replica_groups=rg,
                        ins=[u_shard[0:NL, :]], outs=[u_full[0:N, :]])

                # stats: colsum(u), colsum(u^2) over local valid rows
                lastn = NL - (TN - 1) * 128
                for si in range(2):
                    ps4 = psst.tile([128, 1], f32, tag="st",
                                    name=f"st{si}_{step}")

                    def qsrc(c0, c1, p0=0, p1=128, _si=si):
                        sl = state2[p0:p1, c0:c1]
                        if _si == 0:
                            return sl
                        sqq = smp.tile([128, 128], f32, tag="sqq")
                        nc.vector.tensor_tensor(out=sqq[p0:p1, 0:c1 - c0],
                                                in0=sl, in1=sl,
                                                op=mybir.AluOpType.mult)
                        return sqq[p0:p1, 0:c1 - c0]

                    for jq in range(NQUAD):
                        nc.tensor.matmul(
                            out=ps4[:],
                            lhsT=qsrc(jq * 4 * DH, (jq + 1) * 4 * DH),
                            rhs=ones_sb[:], start=(jq == 0), stop=False)
                    nc.tensor.matmul(out=ps4[0:DH, :],
                                     lhsT=qsrc((TN - 2) * DH, (TN - 1) * DH),
                                     rhs=ones_sb[:],
                                     start=False, stop=False)
                    nc.tensor.matmul(out=ps4[0:DH, :],
                                     lhsT=qsrc((TN - 1) * DH, TN * DH,
                                               0, lastn),
                                     rhs=ones_sb[0:lastn, :],
                                     start=False, stop=True)
                    ps4_sb = smp.tile([128, 1], f32, tag="ps4sb")
                    nc.scalar.activation(out=ps4_sb[:], in_=ps4[:],
                                         func=mybir.ActivationFunctionType.Copy)
                    col = pstr.tile([1, DH], f32, tag="small",
                                    name=f"col{si}_{step}")
                    nc.tensor.matmul(out=col[:], lhsT=ps4_sb[:],
                                     rhs=sel32_sb[:], start=True, stop=True)
                    nc.vector.tensor_copy(
                        out=s_sb[:, si * DH:(si + 1) * DH], in_=col[:])
                nc.sync.dma_start(out=st_in[:], in_=s_sb[:])
                nc.gpsimd.collective_compute(
                    "AllReduce", mybir.AluOpType.add, replica_groups=rg,
                    ins=[st_in[:]], outs=[st_out[:]])
                nc.sync.dma_start(out=stats_sb[:], in_=st_out[:])

                # mean/scale
                mean = smp.tile([1, DH], f32, tag="mean")
                nc.vector.tensor_scalar(
                    out=mean[:], in0=stats_sb[:, 0:DH], scalar1=1.0 / cfg.N,
                    scalar2=None, op0=mybir.AluOpType.mult)
                m2 = smp.tile([1, DH], f32, tag="m2")
                nc.vector.tensor_tensor(out=m2[:], in0=mean[:],
                                        in1=stats_sb[:, 0:DH],
                                        op=mybir.AluOpType.mult)
                r1 = smp.tile([1, 1], f32, tag="r1")
                nc.vector.reduce_sum(out=r1[:], in_=m2[:],
                                     axis=mybir.AxisListType.X)
                qs = smp.tile([1, 1], f32, tag="qs")
                nc.vector.reduce_sum(out=qs[:], in_=stats_sb[:, DH:2 * DH],
                                     axis=mybir.AxisListType.X)
                v_ = smp.tile([1, 1], f32, tag="v_")
                nc.vector.tensor_tensor(out=v_[:], in0=qs[:], in1=r1[:],
                                        op=mybir.AluOpType.subtract)
                nc.vector.tensor_scalar(
                    out=v_[:], in0=v_[:], scalar1=1.0 / cfg.N,
                    scalar2=cfg.EPS, op0=mybir.AluOpType.mult,
                    op1=mybir.AluOpType.add)
                rt = smp.tile([1, 1], f32, tag="rt")
                nc.scalar.activation(out=rt[:], in_=v_[:],
                                     func=mybir.ActivationFunctionType.Sqrt)
                scl = smp.tile([1, 1], f32, tag="scl")
                nc.vector.reciprocal(out=scl[:], in_=rt[:])
                msc = smp.tile([1, DH + 1], f32, tag="msc")
                nc.vector.tensor_copy(out=msc[:, 0:DH], in_=mean[:])
                nc.vector.tensor_copy(out=msc[:, DH:DH + 1], in_=scl[:])
                pmsc = pstr.tile([128, DH + 1], f32, tag="small",
                                 name=f"pmsc{step}")
                nc.tensor.matmul(out=pmsc[:], lhsT=ones_row[0:1, :],
                                 rhs=msc[:], start=True, stop=True)
                msc128 = smp.tile([128, DH + 1], f32, tag="msc128")
                nc.vector.tensor_copy(out=msc128[:], in_=pmsc[:])
                mscb = smp.tile([128, DH + 1], bf16, tag="mscb")
                nc.vector.tensor_copy(out=mscb[:], in_=pmsc[:])

                def meanb(shape):
                    return msc128[:, None, 0:DH].to_broadcast(shape)

                def sclb(shape):
                    return msc128[:, None, DH:DH + 1].to_broadcast(shape)

                def meanbb(shape):
                    return mscb[:, None, 0:DH].to_broadcast(shape)

                def sclbb(shape):
                    return mscb[:, None, DH:DH + 1].to_broadcast(shape)

                # ---- tloc = dinv * relu((u - mean) * scl)  (local rows)
                nc.vector.tensor_tensor(out=state[:], in0=state[:],
                                        in1=meanb([128, TN, DH]),
                                        op=mybir.AluOpType.subtract)
                nc.vector.tensor_tensor(out=state[:], in0=state[:],
                                        in1=sclb([128, TN, DH]),
                                        op=mybir.AluOpType.mult)
                nc.vector.tensor_scalar(out=state[:], in0=state[:],
                                        scalar1=0.0, scalar2=None,
                                        op0=mybir.AluOpType.max)
                nc.vector.tensor_tensor(out=tloc[:], in0=state[:],
                                        in1=dinv_b([128, TN, DH]),
                                        op=mybir.AluOpType.mult)

                # ---- normalize the received full table into tab_n (f32 math)
                SLW = 64
                nslab = math.ceil(TNF / SLW)
                uf_v = u_full.rearrange("(t p) f -> p t f", p=128)
                tn_v = tab_n.rearrange("(t p) f -> p t f", p=128)
                for s in range(nslab if "norm" not in skip else 0):
                    t0s, t1s = s * SLW, min(TNF, (s + 1) * SLW)
                    nt = t1s - t0s
                    us = usl.tile([128, SLW, DH], bf16, tag="uslab")
                    usf = usl.tile([128, SLW, DH], f32, tag="usf")
                    nc.sync.dma_start(out=us[:, 0:nt, :],
                                      in_=uf_v[:, t0s:t1s, :])
                    nc.vector.tensor_copy(out=usf[:, 0:nt, :],
                                          in_=us[:, 0:nt, :])
                    nc.vector.tensor_tensor(out=usf[:, 0:nt, :],
                                            in0=usf[:, 0:nt, :],
                                            in1=meanb([128, nt, DH]),
                                            op=mybir.AluOpType.subtract)
                    nc.vector.tensor_scalar(out=usf[:, 0:nt, :],
                                            in0=usf[:, 0:nt, :],
                                            scalar1=0.0, scalar2=None,
                                            op0=mybir.AluOpType.max)
                    nc.vector.tensor_tensor(out=usf[:, 0:nt, :],
                                            in0=usf[:, 0:nt, :],
                                            in1=sclb([128, nt, DH]),
                                            op=mybir.AluOpType.mult)
                    nc.vector.tensor_tensor(
                        out=us[:, 0:nt, :], in0=usf[:, 0:nt, :],
                        in1=dinvf_sb[:, t0s:t1s, None]
                            .to_broadcast([128, nt, DH]),
                        op=mybir.AluOpType.mult)
                    nc.sync.dma_start(out=tn_v[:, t0s:t1s, :],
                                      in_=us[:, 0:nt, :])

    nc.compile()
    return nc, names


# ----------------------------------------------------------------- entry

def make_inputs(cfg, pp, x, W0, b0, W1, b1, Wf, bf):
    """Per-core input maps for run_bass_kernel_spmd."""
    C, NL, NLP, DH, DC = cfg.C, cfg.NL, cfg.NLP, cfg.DH, cfg.DC
    x_pad = np.zeros((C, NLP, cfg.D_IN), dtype=BF16)
    for c in range(C):
        x_pad[c, :NL] = x[c * NL:(c + 1) * NL].astype(BF16)
    iota_row = np.tile(np.arange(128, dtype=np.float32).astype(BF16)[None, :],
                       (128, 1))
    wblk1 = np.zeros((128, 4 * DH), dtype=np.float32)
    wblkf = np.zeros((128, 4 * DC), dtype=np.float32)
    for s in range(4):
        wblk1[s * DH:(s + 1) * DH, s * DH:(s + 1) * DH] = W1
        wblkf[s * DH:(s + 1) * DH, s * DC:(s + 1) * DC] = Wf
    sel32 = np.zeros((128, DH), dtype=np.float32)
    for s in range(4):
        sel32[s * DH + np.arange(DH), np.arange(DH)] = 1.0
    in_maps = []
    for c in range(C):
        in_maps.append({
            "x_bf": x_pad[c],
            "gidx": pp["gidx"][c],
            "segrel": pp["segrel"][c],
            "dinv_nm": pp["dinv_nm"][c],
            "dinv_full": pp["dinv_full"],
            "w0": W0.astype(BF16),
            "wblk1": wblk1,
            "wblkf": wblkf,
            "sel32": sel32,
            "b0": np.tile(b0.reshape(1, -1).astype(np.float32), (128, 1)),
            "b1q": np.tile(b1.reshape(1, -1).astype(np.float32), (128, 4)),
            "bfq": np.tile(bf.reshape(1, -1).astype(np.float32), (128, 4)),
            "iota_row": iota_row,
        })
    return in_maps


_CACHE = {}

def kernel(x, edge_index, W0, b0, W1, b1, Wf, bf):
    from concourse import bass_utils
    cfg = FULL
    x = np.asarray(x)
    edge_index = np.asarray(edge_index)
    pp = preprocess(cfg, edge_index)
    key = ("prog", pp["RS"])
    if key not in _CACHE:
        _CACHE[key] = build_program(cfg, pp["RS"])
    nc, _names = _CACHE[key]
    in_maps = make_inputs(cfg, pp, x, W0, b0, W1, b1, Wf, bf)
    res = bass_utils.run_bass_kernel_spmd(nc, in_maps, list(range(cfg.C)))
    out = np.concatenate([res.results[c]["out"] for c in range(cfg.C)], axis=0)
    return out.astype(np.float32)
